# revision 6
# baseline (speedup 1.0000x reference)
"""DiscriminativeLoss TRN2 kernel v6 — c-major one-hot, fused phase C, host tail.

Per core: one batch element [N, 32] f32 + labels [N] i32 -> (segsum[32], means[32,32]).
Host finishes: seg_mean, pull_b, and the whole push loss (O(32^2) work).

Layouts (P=128 partitions, npc points/partition, point n = p*npc + c):
  oh_cl  [128, (c, l)] bf16   one-hot, c-major: dense [128,32] weight slices
                              for phase A (fast LDWEIGHTS); built on GPSIMD via
                              broadcast-AP tensor_tensor is_equal.
  hi_st  [128, (c, s)] bf16   32x32-block stream transpose of bf16 emb
  ohT4   [(q,l), m] bf16      transposed one-hot (DMA-replicated labels + TSP)

Phase A (seg sums): 1024 matmuls psum[32l, 32d] += oh_cl_slice^T @ hib_chunk.
  Starts ~3us in (only needs labels + first emb block) -> PE HAM warms early
  and stays warm through phase B.
Phase B (transposed): per slab (ch, s) of 512 cols:
  diff_psum = (-meansT4) @ ohT4_slab + I128 @ embT4_slab   (2 accum matmuls)
  sqd = Square(diff_psum) [ACT] ; d2_psum += ones_s @ sqd  (accum over s)
  hinge: dist = sqrt(d2+eps), h = relu(dist - dv)          [ACT]
Phase C: segsum[l] col via ONE scalar_tensor_tensor(oh*h, accum_out) per
  (ch,l), split DVE/GPSIMD; then 2 accumulating matmuls cross-partition.
Tail: copy segsum-total + means into [32,33] and DMA out. Push loss on host.
"""

import sys

sys.path.insert(0, "/opt/trn_rl_repo")

import numpy as np
from contextlib import ExitStack

import concourse.bass as bass
import concourse.bacc as bacc
import concourse.mybir as mybir
import concourse.tile as tile

F32 = mybir.dt.float32
BF16 = mybir.dt.bfloat16
I32 = mybir.dt.int32
AX = mybir.AxisListType
OP = mybir.AluOpType
AF = mybir.ActivationFunctionType

D = 32
NL = 32          # instance labels 1..32 (label 0 ignored everywhere)
SQ = 32          # partitions per quarter
DELTA_V = 0.1
DELTA_D = 0.5


def emit(tc, emb_d, lab16_d, cnt_d, res_d, npc):
    nc = tc.nc
    ctx = tc.ctx
    P = 128
    H = npc // 2          # cols per ch-half (per s)
    SQH = SQ * H          # embT4 cols per ch
    NBLK = npc // 32      # 32-chunk blocks

    emb_v = emb_d[:].rearrange("(p c) d -> p (c d)", p=P)
    lab_v = lab16_d[:].rearrange("(p c) -> p c", p=P)

    # ---------------- pools ----------------
    p_pers = ctx.enter_context(tc.tile_pool(name="p_pers", bufs=1))
    p_small = ctx.enter_context(tc.tile_pool(name="p_small", bufs=1))
    p_t2 = ctx.enter_context(tc.tile_pool(name="p_t2", bufs=7))   # epoch2 tiles
    p_junk = ctx.enter_context(tc.tile_pool(name="p_junk", bufs=2))
    ps_misc = ctx.enter_context(tc.tile_pool(name="ps_misc", bufs=1, space="PSUM"))

    # ---------------- persistent ----------------
    oh_cl = p_pers.tile([P, npc * NL], BF16, tag="oh_cl")
    ohc3 = oh_cl[:].rearrange("p (c l) -> p c l", l=NL)
    # hi_st = 32x32-block stream transpose of hi: hi_st[32q+d, 32c+s] =
    # bf16 emb of point (q,s,c), dim d.  Slab (ch, s) reads cols
    # {32*(ch*H+cc)+s} — a strided AP, so no shuffle DMA is needed.
    hi_st = p_pers.tile([P, npc * D], BF16, tag="hi_st")
    hst3 = hi_st[:].rearrange("p (c s) -> p c s", s=SQ)
    h_all = p_pers.tile([P, npc], BF16, tag="h_all")
    lab_b = p_pers.tile([P, npc], BF16, tag="lab_b")
    segsum = p_pers.tile([P, 2 * NL], F32, tag="segsum")

    # ---------------- small constants ----------------
    negmT4 = p_small.tile([P, P], BF16, tag="negmT4")
    nc.vector.memset(negmT4[:], 0.0)
    id128 = p_small.tile([P, P], BF16, tag="id128")
    ones128b = p_small.tile([P, P], BF16, tag="ones128b")
    nc.vector.memset(ones128b[:], 1.0)
    nc.gpsimd.affine_select(
        id128[:], ones128b[:], pattern=[[1, P]], base=0,
        channel_multiplier=-1, compare_op=OP.is_equal, fill=0.0,
    )
    # ones_s base: base[r, c] = 1 iff c == 32*(r//32) + 31; view offset 31-s
    ones_base = p_small.tile([P, P + SQ], BF16, tag="ones_base")
    nc.vector.memset(ones_base[:], 0.0)
    for q in range(4):
        nc.vector.memset(ones_base[SQ * q:SQ * (q + 1), SQ * q + 31:SQ * q + 32], 1.0)
    ones128f = p_small.tile([P, 1], F32, tag="ones128f")
    nc.vector.memset(ones128f[:], 1.0)
    # lvec: value (p % 32) + 1 per partition
    lvec_i = p_small.tile([P, 1], I32, tag="lvec_i")
    nc.gpsimd.iota(lvec_i[:], pattern=[[0, 1]], base=0, channel_multiplier=1)
    lvec_m = p_small.tile([P, 1], I32, tag="lvec_m")
    nc.vector.tensor_scalar(out=lvec_m[:], in0=lvec_i[:], scalar1=31,
                            scalar2=None, op0=OP.bitwise_and)
    lvec_f = p_small.tile([P, 1], F32, tag="lvec_f")
    nc.vector.tensor_copy(lvec_f[:], lvec_m[:])
    lvec = p_small.tile([P, 1], F32, tag="lvec")
    nc.vector.tensor_scalar(out=lvec[:], in0=lvec_f[:], scalar1=1.0,
                            scalar2=None, op0=OP.add)
    # lrow: row vector of labels 1..32 on every partition (for oh_cl build)
    lrow_i = p_small.tile([P, NL], I32, tag="lrow_i")
    nc.gpsimd.iota(lrow_i[:], pattern=[[1, NL]], base=1, channel_multiplier=0)
    lrow = p_small.tile([P, NL], BF16, tag="lrow")
    nc.vector.tensor_copy(lrow[:], lrow_i[:])
    eps_b = p_small.tile([P, 1], F32, tag="eps_b")
    nc.vector.memset(eps_b[:], 1e-24)
    ndv_b = p_small.tile([P, 1], F32, tag="ndv_b")
    nc.vector.memset(ndv_b[:], -DELTA_V)

    # ---------------- labels (bf16 from host) + counts (host bincount) ----
    nc.sync.dma_start(lab_b[:], lab_v)
    cnt_sb = p_small.tile([32, 1], F32, tag="cnt_sb")
    nc.sync.dma_start(cnt_sb[:], cnt_d[:].rearrange("(l o) -> l o", o=1))

    ps_a = tc.alloc_tile_pool(name="ps_a", bufs=1, space="PSUM")
    psum_a = ps_a.tile([32, D], F32, tag="psum_a")

    # one-hot c-major, built per block on GPSIMD: oh_cl[p, 32c+l] =
    # (lab[p,c] == l+1).  in0 = lab broadcast over l, in1 = lrow broadcast
    # over c (both 0-stride APs).
    lab3 = lab_b[:].unsqueeze(2)
    lrow3 = lrow[:].unsqueeze(1)
    for b in range(NBLK):
        nc.vector.tensor_tensor(
            out=ohc3[:, b * 32:(b + 1) * 32, :],
            in0=lab3[:, b * 32:(b + 1) * 32, :].broadcast_to([P, 32, NL]),
            in1=lrow3.broadcast_to([P, 32, NL]),
            op=OP.is_equal)

    # ================= EPOCH 1: stream emb, phase A + block transpose =======
    with tc.tile_pool(name="p_in", bufs=4) as p_in, \
         tc.tile_pool(name="p_hib", bufs=4) as p_hib:
        for b in range(NBLK):
            ta = p_in.tile([P, 1024], F32, tag="ta")
            nc.sync.dma_start(ta[:], emb_v[:, b * 1024:(b + 1) * 1024])
            hib = p_hib.tile([P, 1024], BF16, tag="hib")
            nc.scalar.copy(hib[:], ta[:])
            nc.vector.transpose(hi_st[:, b * 1024:(b + 1) * 1024], hib[:])
            for j in range(32):
                c = b * 32 + j
                nc.tensor.matmul(
                    psum_a[:], oh_cl[:, c * NL:(c + 1) * NL],
                    hib[:, j * D:(j + 1) * D],
                    start=(c == 0), stop=(c == npc - 1),
                )

    # pre-warm epoch-2 inputs that do not depend on the means
    labdr3 = lab16_d[:].rearrange("(q s c) -> q s c", q=4, s=SQ)
    prewarm = {}
    for s in range(4):
        labT = p_t2.tile([P, H], BF16, tag="labT")
        nc.sync.dma_start(
            labT[:],
            labdr3[:, s, 0:H].unsqueeze(1).broadcast_to([4, SQ, H]),
        )
        ohT = p_t2.tile([P, H], BF16, tag="ohT")
        nc.vector.tensor_scalar(out=ohT[:], in0=labT[:], scalar1=lvec[:],
                                scalar2=None, op0=OP.is_equal)
        prewarm[s] = ohT

    # ================= means =================
    cnt_cl = p_small.tile([32, 1], F32, tag="cnt_cl")
    nc.vector.tensor_scalar(out=cnt_cl[:], in0=cnt_sb[:], scalar1=1.0,
                            scalar2=None, op0=OP.max)
    recip = p_small.tile([32, 1], F32, tag="recip")
    nc.vector.reciprocal(recip[:], cnt_cl[:])
    nrecip = p_small.tile([32, 1], F32, tag="nrecip")
    nc.vector.tensor_scalar(out=nrecip[:], in0=recip[:], scalar1=-1.0,
                            scalar2=None, op0=OP.mult)
    means_f = p_small.tile([32, 32], F32, tag="means_f")
    nc.vector.tensor_scalar(out=means_f[:], in0=psum_a[:], scalar1=recip[:],
                            scalar2=None, op0=OP.mult)
    negm_b = p_small.tile([32, 32], BF16, tag="negm_b")
    nc.scalar.activation(negm_b[:], psum_a[:], AF.Copy, scale=nrecip[:])
    # replicate into 4 diagonal blocks of negmT4 (partition-shifting DMAs)
    for q in range(4):
        nc.sync.dma_start(negmT4[SQ * q:SQ * (q + 1), SQ * q:SQ * q + 32],
                          negm_b[:])
    ps_a.release()

    # ================= EPOCH 2: transposed phase B + phase C ================
    ps_diff = tc.alloc_tile_pool(name="ps_diff", bufs=4, space="PSUM")
    ps_d2 = tc.alloc_tile_pool(name="ps_d2", bufs=1, space="PSUM")
    d2_bank = {}
    for ch in range(2):
        d2_bank[ch] = ps_d2.tile([P, H], F32, tag=f"d2_{ch}", name=f"d2_{ch}")
    LAG = 5   # d2-mm trails its slab: the in-order PE never waits on ACT
    sqd_q = {}
    for ch in range(2):
        for s in range(SQ):
            if ch == 0 and s in prewarm:
                ohT = prewarm[s]
            else:
                labT = p_t2.tile([P, H], BF16, tag="labT")
                nc.sync.dma_start(
                    labT[:],
                    labdr3[:, s, ch * H:(ch + 1) * H]
                    .unsqueeze(1).broadcast_to([4, SQ, H]),
                )
                ohT = p_t2.tile([P, H], BF16, tag="ohT")
                nc.vector.tensor_scalar(out=ohT[:], in0=labT[:],
                                        scalar1=lvec[:],
                                        scalar2=None, op0=OP.is_equal)
            dpsum = ps_diff.tile([P, H], F32, tag="dpsum")
            nc.tensor.matmul(dpsum[:], negmT4[:], ohT[:], start=True, stop=False)
            nc.tensor.matmul(dpsum[:], id128[:], hst3[:, ch * H:(ch + 1) * H, s],
                             start=False, stop=True)
            sqd = p_t2.tile([P, H], BF16, tag="sqd")
            nc.scalar.activation(sqd[:], dpsum[:], AF.Square)
            sqd_q[s] = sqd
            if s >= LAG:
                sp = s - LAG
                nc.tensor.matmul(d2_bank[ch][:],
                                 ones_base[:, 31 - sp:159 - sp],
                                 sqd_q.pop(sp)[:], start=(sp == 0), stop=False,
                                 skip_group_check=True)
        for sp in sorted(sqd_q):
            nc.tensor.matmul(d2_bank[ch][:], ones_base[:, 31 - sp:159 - sp],
                             sqd_q[sp][:], start=(sp == 0), stop=(sp == SQ - 1),
                             skip_group_check=True)
        sqd_q.clear()
        dist = p_t2.tile([P, H], F32, tag="dist")
        nc.scalar.activation(dist[:], d2_bank[ch][:], AF.Sqrt, bias=eps_b[:])
        nc.scalar.activation(h_all[:, ch * H:(ch + 1) * H], dist[:],
                             AF.Relu, bias=ndv_b[:])
        # phase C: fused (oh * h) multiply + per-partition reduce, one instr
        # per l, split DVE / GPSIMD.  oh read is a stride-32 AP (c-major).
        for l in range(NL):
            junk = p_junk.tile([P, H], BF16, tag="junk")
            col = segsum[:, ch * NL + l:ch * NL + l + 1]
            nc.vector.scalar_tensor_tensor(
                out=junk[:], in0=ohc3[:, ch * H:(ch + 1) * H, l],
                scalar=1.0, in1=h_all[:, ch * H:(ch + 1) * H],
                op0=OP.mult, op1=OP.mult, accum_out=col)

    # phase C cross-partition reduce: two accumulating matmuls
    ps_seg = ps_misc.tile([32, 1], F32, tag="misc")
    nc.tensor.matmul(ps_seg[:], segsum[:, 0:NL], ones128f[:],
                     start=True, stop=False)
    nc.tensor.matmul(ps_seg[:], segsum[:, NL:2 * NL], ones128f[:],
                     start=False, stop=True)

    # ================= export: [32, 33] = [segsum | means] =================
    res_sb = p_small.tile([32, 1 + 32], F32, tag="res_sb")
    nc.vector.tensor_copy(res_sb[:, 0:1], ps_seg[:])
    nc.vector.tensor_copy(res_sb[:, 1:33], means_f[:])
    nc.sync.dma_start(res_d[:], res_sb[:])
    ps_d2.release()
    ps_diff.release()


def build_program(npc):
    n = npc * 128
    nc = bacc.Bacc("TRN2", target_bir_lowering=False, debug=False)
    emb_d = nc.dram_tensor("emb", [n, D], F32, kind="ExternalInput")
    lab16_d = nc.dram_tensor("lab16", [n], BF16, kind="ExternalInput")
    cnt_d = nc.dram_tensor("cnt", [32], F32, kind="ExternalInput")
    res_d = nc.dram_tensor("res", [32, 33], F32, kind="ExternalOutput")
    with tile.TileContext(nc) as tc:
        with ExitStack() as ctx:
            tc.ctx = ctx
            emit(tc, emb_d, lab16_d, cnt_d, res_d, npc)
    nc.compile()
    return nc


_NC_CACHE = {}


def _get_nc(npc):
    if npc not in _NC_CACHE:
        _NC_CACHE[npc] = build_program(npc)
    return _NC_CACHE[npc]


def kernel(embeddings, labels):
    embeddings = np.asarray(embeddings, dtype=np.float32)
    labels = np.asarray(labels, dtype=np.int32)
    bsz = embeddings.shape[0]
    npc = embeddings.shape[1] // 128
    nc = _get_nc(npc)

    from concourse.bass_utils import run_bass_kernel_spmd

    import ml_dtypes
    lab16 = labels.astype(np.float32).astype(ml_dtypes.bfloat16)
    counts = np.stack([
        np.bincount(labels[b], minlength=33)[1:33].astype(np.float32)
        for b in range(bsz)
    ])
    in_maps = [
        {"emb": np.ascontiguousarray(embeddings[b]),
         "lab16": np.ascontiguousarray(lab16[b]),
         "cnt": counts[b]}
        for b in range(bsz)
    ]
    out = run_bass_kernel_spmd(nc, in_maps, list(range(bsz)))

    pull_bs = np.zeros(bsz, dtype=np.float32)
    push_bs = np.zeros(bsz, dtype=np.float32)
    for b in range(bsz):
        res = np.asarray(out.results[b]["res"], dtype=np.float32)  # [32, 33]
        seg = res[:, 0]
        means = res[:, 1:33]
        cnt = counts[b]
        cntc = np.maximum(cnt, 1.0)
        seg_mean = seg / cntc
        present = cnt > 0
        n_inst = np.float32(present.sum())
        pull_bs[b] = seg_mean.sum() / (n_inst + np.float32(1e-6))
        # push: hinge margin between normalized instance means
        nrm = np.sqrt((means * means).sum(-1))
        mn = means / np.maximum(nrm, 1e-12)[:, None]
        sq = ((mn[:, None, :] - mn[None, :, :]) ** 2).sum(-1)
        dmat = np.sqrt(sq + 1e-24)
        K = NL
        triu = np.triu(np.ones((K, K), np.float32), 1)
        pmask = triu * present[:, None] * present[None, :]
        hp = np.maximum(2.0 * DELTA_D - dmat, 0.0) * pmask
        if n_inst > 1:
            push_bs[b] = hp.sum() / (pmask.sum() + np.float32(1e-6))
        else:
            push_bs[b] = 0.0
    pull = pull_bs.sum() / bsz
    push = push_bs.sum() / bsz
    return np.stack([pull + push, pull, push]).astype(np.float32)


# revision 11
# speedup vs baseline: 1.3568x; 1.3568x over previous
"""DiscriminativeLoss TRN2 kernel v6 — c-major one-hot, fused phase C, host tail.

Per core: one batch element [N, 32] f32 + labels [N] i32 -> (segsum[32], means[32,32]).
Host finishes: seg_mean, pull_b, and the whole push loss (O(32^2) work).

Layouts (P=128 partitions, npc points/partition, point n = p*npc + c):
  oh_cl  [128, (c, l)] bf16   one-hot, c-major: dense [128,32] weight slices
                              for phase A (fast LDWEIGHTS); built on GPSIMD via
                              broadcast-AP tensor_tensor is_equal.
  hi_st  [128, (c, s)] bf16   32x32-block stream transpose of bf16 emb
  ohT4   [(q,l), m] bf16      transposed one-hot (DMA-replicated labels + TSP)

Phase A (seg sums): 1024 matmuls psum[32l, 32d] += oh_cl_slice^T @ hib_chunk.
  Starts ~3us in (only needs labels + first emb block) -> PE HAM warms early
  and stays warm through phase B.
Phase B (transposed): per slab (ch, s) of 512 cols:
  diff_psum = (-meansT4) @ ohT4_slab + I128 @ embT4_slab   (2 accum matmuls)
  sqd = Square(diff_psum) [ACT] ; d2_psum += ones_s @ sqd  (accum over s)
  hinge: dist = sqrt(d2+eps), h = relu(dist - dv)          [ACT]
Phase C: segsum[l] col via ONE scalar_tensor_tensor(oh*h, accum_out) per
  (ch,l), split DVE/GPSIMD; then 2 accumulating matmuls cross-partition.
Tail: copy segsum-total + means into [32,33] and DMA out. Push loss on host.
"""

import sys

sys.path.insert(0, "/opt/trn_rl_repo")

import numpy as np
from contextlib import ExitStack

import concourse.bass as bass
import concourse.bacc as bacc
import concourse.mybir as mybir
import concourse.tile as tile

F32 = mybir.dt.float32
BF16 = mybir.dt.bfloat16
I32 = mybir.dt.int32
AX = mybir.AxisListType
OP = mybir.AluOpType
AF = mybir.ActivationFunctionType

D = 32
NL = 32          # instance labels 1..32 (label 0 ignored everywhere)
SQ = 32          # partitions per quarter
DELTA_V = 0.1
DELTA_D = 0.5


def emit(tc, emb_d, lab16_d, cnt_d, res_d, npc):
    nc = tc.nc
    ctx = tc.ctx
    P = 128
    H = npc // 2          # cols per ch-half (per s)
    SQH = SQ * H          # embT4 cols per ch
    NBLK = npc // 32      # 32-chunk blocks

    emb_v = emb_d[:].rearrange("(p c) d -> p (c d)", p=P)
    lab_v = lab16_d[:].rearrange("(p c) -> p c", p=P)

    # ---------------- pools ----------------
    p_pers = ctx.enter_context(tc.tile_pool(name="p_pers", bufs=1))
    p_small = ctx.enter_context(tc.tile_pool(name="p_small", bufs=1))
    p_t2 = ctx.enter_context(tc.tile_pool(name="p_t2", bufs=7))   # epoch2 tiles
    p_junk = ctx.enter_context(tc.tile_pool(name="p_junk", bufs=2))
    ps_misc = ctx.enter_context(tc.tile_pool(name="ps_misc", bufs=1, space="PSUM"))

    # ---------------- persistent ----------------
    oh_lj = p_pers.tile([P, NL * npc], BF16, tag="oh_lj")
    oh3 = oh_lj[:].rearrange("p (l c) -> p l c", c=npc)
    # hi_st = 32x32-block stream transpose of hi: hi_st[32q+d, 32c+s] =
    # bf16 emb of point (q,s,c), dim d.  Slab (ch, s) reads cols
    # {32*(ch*H+cc)+s} — a strided AP, so no shuffle DMA is needed.
    hi_st = p_pers.tile([P, npc * D], BF16, tag="hi_st")
    hst3 = hi_st[:].rearrange("p (c s) -> p c s", s=SQ)
    h_all = p_pers.tile([P, npc], BF16, tag="h_all")
    lab_b = p_pers.tile([P, npc], BF16, tag="lab_b")
    segsum = p_pers.tile([P, 2 * NL], F32, tag="segsum")

    # ---------------- small constants ----------------
    negmT4 = p_small.tile([P, P], BF16, tag="negmT4")
    nc.vector.memset(negmT4[:], 0.0)
    id128 = p_small.tile([P, P], BF16, tag="id128")
    ones128b = p_small.tile([P, P], BF16, tag="ones128b")
    nc.vector.memset(ones128b[:], 1.0)
    nc.gpsimd.affine_select(
        id128[:], ones128b[:], pattern=[[1, P]], base=0,
        channel_multiplier=-1, compare_op=OP.is_equal, fill=0.0,
    )
    # ones_s base: base[r, c] = 1 iff c == 32*(r//32) + 31; view offset 31-s
    ones_base = p_small.tile([P, P + SQ], BF16, tag="ones_base")
    nc.vector.memset(ones_base[:], 0.0)
    for q in range(4):
        nc.vector.memset(ones_base[SQ * q:SQ * (q + 1), SQ * q + 31:SQ * q + 32], 1.0)
    ones128f = p_small.tile([P, 1], F32, tag="ones128f")
    nc.vector.memset(ones128f[:], 1.0)
    # lvec: value (p % 32) + 1 per partition
    lvec_i = p_small.tile([P, 1], I32, tag="lvec_i")
    nc.gpsimd.iota(lvec_i[:], pattern=[[0, 1]], base=0, channel_multiplier=1)
    lvec_m = p_small.tile([P, 1], I32, tag="lvec_m")
    nc.vector.tensor_scalar(out=lvec_m[:], in0=lvec_i[:], scalar1=31,
                            scalar2=None, op0=OP.bitwise_and)
    lvec_f = p_small.tile([P, 1], F32, tag="lvec_f")
    nc.vector.tensor_copy(lvec_f[:], lvec_m[:])
    lvec = p_small.tile([P, 1], F32, tag="lvec")
    nc.vector.tensor_scalar(out=lvec[:], in0=lvec_f[:], scalar1=1.0,
                            scalar2=None, op0=OP.add)
    eps_b = p_small.tile([P, 1], F32, tag="eps_b")
    nc.vector.memset(eps_b[:], 1e-24)
    ndv_b = p_small.tile([P, 1], F32, tag="ndv_b")
    nc.vector.memset(ndv_b[:], -DELTA_V)

    # ---------------- labels (bf16 from host) + counts (host bincount) ----
    nc.sync.dma_start(lab_b[:], lab_v)
    cnt_sb = p_small.tile([32, 1], F32, tag="cnt_sb")
    nc.sync.dma_start(cnt_sb[:], cnt_d[:].rearrange("(l o) -> l o", o=1))

    ps_a = tc.alloc_tile_pool(name="ps_a", bufs=1, space="PSUM")
    psum_a = ps_a.tile([32, D], F32, tag="psum_a")

    # one-hot rows (l-major), built in graded c-segments so the first
    # phase-A matmul only waits on a small segment (fast start -> HAM warm).
    segs = [0, npc // 16, npc // 4, npc // 2, npc]
    for g in range(len(segs) - 1):
        lo, hi = segs[g], segs[g + 1]
        for l in range(NL):
            nc.vector.tensor_scalar(out=oh3[:, l, lo:hi], in0=lab_b[:, lo:hi],
                                    scalar1=float(l + 1), scalar2=None,
                                    op0=OP.is_equal)

    # ================= EPOCH 1: stream emb, phase A + block transpose =======
    with tc.tile_pool(name="p_in", bufs=4) as p_in, \
         tc.tile_pool(name="p_hib", bufs=4) as p_hib:
        for b in range(NBLK):
            ta = p_in.tile([P, 1024], F32, tag="ta")
            nc.sync.dma_start(ta[:], emb_v[:, b * 1024:(b + 1) * 1024])
            hib = p_hib.tile([P, 1024], BF16, tag="hib")
            nc.scalar.copy(hib[:], ta[:])
            nc.vector.transpose(hi_st[:, b * 1024:(b + 1) * 1024], hib[:])
            for j in range(32):
                c = b * 32 + j
                nc.tensor.matmul(
                    psum_a[:], oh3[:, :, c],
                    hib[:, j * D:(j + 1) * D],
                    start=(c == 0), stop=(c == npc - 1),
                )

    # pre-warm epoch-2 inputs that do not depend on the means
    labdr3 = lab16_d[:].rearrange("(q s c) -> q s c", q=4, s=SQ)
    prewarm = {}
    for s in range(4):
        labT = p_t2.tile([P, H], BF16, tag="labT")
        nc.sync.dma_start(
            labT[:],
            labdr3[:, s, 0:H].unsqueeze(1).broadcast_to([4, SQ, H]),
        )
        ohT = p_t2.tile([P, H], BF16, tag="ohT")
        nc.vector.tensor_scalar(out=ohT[:], in0=labT[:], scalar1=lvec[:],
                                scalar2=None, op0=OP.is_equal)
        prewarm[s] = ohT

    # ================= means =================
    cnt_cl = p_small.tile([32, 1], F32, tag="cnt_cl")
    nc.vector.tensor_scalar(out=cnt_cl[:], in0=cnt_sb[:], scalar1=1.0,
                            scalar2=None, op0=OP.max)
    recip = p_small.tile([32, 1], F32, tag="recip")
    nc.vector.reciprocal(recip[:], cnt_cl[:])
    nrecip = p_small.tile([32, 1], F32, tag="nrecip")
    nc.vector.tensor_scalar(out=nrecip[:], in0=recip[:], scalar1=-1.0,
                            scalar2=None, op0=OP.mult)
    means_f = p_small.tile([32, 32], F32, tag="means_f")
    nc.vector.tensor_scalar(out=means_f[:], in0=psum_a[:], scalar1=recip[:],
                            scalar2=None, op0=OP.mult)
    negm_b = p_small.tile([32, 32], BF16, tag="negm_b")
    nc.scalar.activation(negm_b[:], psum_a[:], AF.Copy, scale=nrecip[:])
    # replicate into 4 diagonal blocks of negmT4 (partition-shifting DMAs)
    for q in range(4):
        nc.sync.dma_start(negmT4[SQ * q:SQ * (q + 1), SQ * q:SQ * q + 32],
                          negm_b[:])
    ps_a.release()

    # ================= EPOCH 2: transposed phase B + phase C ================
    ps_diff = tc.alloc_tile_pool(name="ps_diff", bufs=4, space="PSUM")
    ps_d2 = tc.alloc_tile_pool(name="ps_d2", bufs=1, space="PSUM")
    d2_bank = {}
    for ch in range(2):
        d2_bank[ch] = ps_d2.tile([P, H], F32, tag=f"d2_{ch}", name=f"d2_{ch}")
    LAG = 5   # d2-mm trails its slab: the in-order PE never waits on ACT
    sqd_q = {}
    for ch in range(2):
        for s in range(SQ):
            if ch == 0 and s in prewarm:
                ohT = prewarm[s]
            else:
                labT = p_t2.tile([P, H], BF16, tag="labT")
                nc.sync.dma_start(
                    labT[:],
                    labdr3[:, s, ch * H:(ch + 1) * H]
                    .unsqueeze(1).broadcast_to([4, SQ, H]),
                )
                ohT = p_t2.tile([P, H], BF16, tag="ohT")
                nc.vector.tensor_scalar(out=ohT[:], in0=labT[:],
                                        scalar1=lvec[:],
                                        scalar2=None, op0=OP.is_equal)
            dpsum = ps_diff.tile([P, H], F32, tag="dpsum")
            nc.tensor.matmul(dpsum[:], negmT4[:], ohT[:], start=True, stop=False)
            nc.tensor.matmul(dpsum[:], id128[:], hst3[:, ch * H:(ch + 1) * H, s],
                             start=False, stop=True)
            sqd = p_t2.tile([P, H], BF16, tag="sqd")
            nc.scalar.activation(sqd[:], dpsum[:], AF.Square)
            sqd_q[s] = sqd
            if s >= LAG:
                sp = s - LAG
                nc.tensor.matmul(d2_bank[ch][:],
                                 ones_base[:, 31 - sp:159 - sp],
                                 sqd_q.pop(sp)[:], start=(sp == 0), stop=False,
                                 skip_group_check=True)
        for sp in sorted(sqd_q):
            nc.tensor.matmul(d2_bank[ch][:], ones_base[:, 31 - sp:159 - sp],
                             sqd_q[sp][:], start=(sp == 0), stop=(sp == SQ - 1),
                             skip_group_check=True)
        sqd_q.clear()
        dist = p_t2.tile([P, H], F32, tag="dist")
        nc.scalar.activation(dist[:], d2_bank[ch][:], AF.Sqrt, bias=eps_b[:])
        nc.scalar.activation(h_all[:, ch * H:(ch + 1) * H], dist[:],
                             AF.Relu, bias=ndv_b[:])
        # phase C: fused (oh * h) multiply + per-partition reduce, one instr
        # per l, split DVE / GPSIMD.  oh read is a stride-32 AP (c-major).
        for l in range(NL):
            junk = p_junk.tile([P, H], BF16, tag="junk")
            col = segsum[:, ch * NL + l:ch * NL + l + 1]
            nc.vector.scalar_tensor_tensor(
                out=junk[:], in0=oh3[:, l, ch * H:(ch + 1) * H],
                scalar=1.0, in1=h_all[:, ch * H:(ch + 1) * H],
                op0=OP.mult, op1=OP.mult, accum_out=col)

    # phase C cross-partition reduce: two accumulating matmuls
    ps_seg = ps_misc.tile([32, 1], F32, tag="misc")
    nc.tensor.matmul(ps_seg[:], segsum[:, 0:NL], ones128f[:],
                     start=True, stop=False)
    nc.tensor.matmul(ps_seg[:], segsum[:, NL:2 * NL], ones128f[:],
                     start=False, stop=True)

    # ================= export: [32, 33] = [segsum | means] =================
    res_sb = p_small.tile([32, 1 + 32], F32, tag="res_sb")
    nc.vector.tensor_copy(res_sb[:, 0:1], ps_seg[:])
    nc.vector.tensor_copy(res_sb[:, 1:33], means_f[:])
    nc.sync.dma_start(res_d[:], res_sb[:])
    ps_d2.release()
    ps_diff.release()


def build_program(npc):
    n = npc * 128
    nc = bacc.Bacc("TRN2", target_bir_lowering=False, debug=False)
    emb_d = nc.dram_tensor("emb", [n, D], F32, kind="ExternalInput")
    lab16_d = nc.dram_tensor("lab16", [n], BF16, kind="ExternalInput")
    cnt_d = nc.dram_tensor("cnt", [32], F32, kind="ExternalInput")
    res_d = nc.dram_tensor("res", [32, 33], F32, kind="ExternalOutput")
    with tile.TileContext(nc) as tc:
        with ExitStack() as ctx:
            tc.ctx = ctx
            emit(tc, emb_d, lab16_d, cnt_d, res_d, npc)
    nc.compile()
    return nc


_NC_CACHE = {}


def _get_nc(npc):
    if npc not in _NC_CACHE:
        _NC_CACHE[npc] = build_program(npc)
    return _NC_CACHE[npc]


def kernel(embeddings, labels):
    embeddings = np.asarray(embeddings, dtype=np.float32)
    labels = np.asarray(labels, dtype=np.int32)
    bsz = embeddings.shape[0]
    npc = embeddings.shape[1] // 128
    nc = _get_nc(npc)

    from concourse.bass_utils import run_bass_kernel_spmd

    import ml_dtypes
    lab16 = labels.astype(np.float32).astype(ml_dtypes.bfloat16)
    counts = np.stack([
        np.bincount(labels[b], minlength=33)[1:33].astype(np.float32)
        for b in range(bsz)
    ])
    in_maps = [
        {"emb": np.ascontiguousarray(embeddings[b]),
         "lab16": np.ascontiguousarray(lab16[b]),
         "cnt": counts[b]}
        for b in range(bsz)
    ]
    out = run_bass_kernel_spmd(nc, in_maps, list(range(bsz)))

    pull_bs = np.zeros(bsz, dtype=np.float32)
    push_bs = np.zeros(bsz, dtype=np.float32)
    for b in range(bsz):
        res = np.asarray(out.results[b]["res"], dtype=np.float32)  # [32, 33]
        seg = res[:, 0]
        means = res[:, 1:33]
        cnt = counts[b]
        cntc = np.maximum(cnt, 1.0)
        seg_mean = seg / cntc
        present = cnt > 0
        n_inst = np.float32(present.sum())
        pull_bs[b] = seg_mean.sum() / (n_inst + np.float32(1e-6))
        # push: hinge margin between normalized instance means
        nrm = np.sqrt((means * means).sum(-1))
        mn = means / np.maximum(nrm, 1e-12)[:, None]
        sq = ((mn[:, None, :] - mn[None, :, :]) ** 2).sum(-1)
        dmat = np.sqrt(sq + 1e-24)
        K = NL
        triu = np.triu(np.ones((K, K), np.float32), 1)
        pmask = triu * present[:, None] * present[None, :]
        hp = np.maximum(2.0 * DELTA_D - dmat, 0.0) * pmask
        if n_inst > 1:
            push_bs[b] = hp.sum() / (pmask.sum() + np.float32(1e-6))
        else:
            push_bs[b] = 0.0
    pull = pull_bs.sum() / bsz
    push = push_bs.sum() / bsz
    return np.stack([pull + push, pull, push]).astype(np.float32)
